# revision 1
# baseline (speedup 1.0000x reference)
"""GNN message-passing (2x GAT + 2x GIN, 2 edge types) on 8 trn2 NeuronCores.

Sharding: cores 0-3 handle edge type 0, cores 4-7 edge type 1 (independent
quads). Within a quad, nodes are sharded by dst range (12500/core, padded to
12544); each edge lives on the core owning its dst, bucketed by src-quarter so
gather indices are quarter-local (< 32768, fits int16).

Per GAT layer: each core computes z/el/er for its node shard on PE (el/er come
free as extra matmul columns using host-precomputed W@al / W@ar), packs
[z|el|er] into bf16 rows, AllGathers within the quad, then per edge chunk:
dma_gather of packed src rows + dma_gather of (el,er) dst rows, edge softmax
without segment-max (logits are O(1) so exp is safe; exp(e)/sum(exp(e)) is
mathematically identical to the max-subtracted form), dma_scatter_add of
[w*z | w] f32 into the local dst accumulator, then normalize num/den locally.

GIN layers: bf16 gathers/scatter-adds of neighbor rows, feature-major MLP on
PE, batchnorm stats per-partition + a tiny quad AllReduce.

Scatter accumulators are ExternalOutputs so the runtime pre-zeroes them.
"""

import sys

for _p in ("/opt/trn_rl_repo",):
    if _p not in sys.path:
        sys.path.insert(0, _p)

import numpy as np
import ml_dtypes

import concourse.bacc as bacc
import concourse.bass as bass
import concourse.tile as tile
import concourse.mybir as mybir
from concourse.bass_utils import run_bass_kernel_spmd

FP32 = mybir.dt.float32
BF16 = mybir.dt.bfloat16
I16 = mybir.dt.int16
AF = mybir.ActivationFunctionType
ALU = mybir.AluOpType

# problem constants
N, IN, HID, H, D = 50000, 128, 256, 4, 64
E, T = 400000, 2
BN_EPS = 1e-5
P = 4                     # cores per quad
NQ = N // P               # real nodes per core: 12500
NCP = 12544               # padded (98 * 128); dump row = NQ
NT = NCP // 128           # 98 node tiles
ZW = 384                  # packed zel row: [z 256 | el 4 | er 4 | pad]
CE = 1024                 # edge-chunk tokens (small: SWDGE ring safety)
SLOTS = CE // 128         # 8
ACC_W = 320               # GAT accumulator row: [num 256 | den 4 | junk]
RGROUPS = [[0, 1, 2, 3], [4, 5, 6, 7]]
DEBUG_TAPS = False
EDGE_DBG = 0  # 1=no scatter, 2=+no er-gather, 3=+no exp/msg
STAGES = 99   # bisect knob: 1=gat_node0 2=+AG 3=+edge 4=+post 5=+gat1 6=+gin0 7=all


def _bf(x):
    return np.asarray(x, dtype=ml_dtypes.bfloat16)


def _wrap_idx(a):
    """[CE] ints -> [128, CE//16] int16 SWDGE wrapped layout (token i at
    [i % 16, i // 16], replicated across the 8 Q7 cores)."""
    w = a.reshape(CE // 16, 16).T.astype(np.int16)
    return np.tile(w, (8, 1))


def _rank_sort(ss, dd):
    """Sort edges by (rank-within-dst, dst). Within a rank slice every dst is
    unique, which makes dma_scatter_add race-free per instruction."""
    order = np.argsort(dd, kind="stable")
    ds = dd[order]
    n = len(ds)
    if n == 0:
        return ss[:0], dd[:0], np.zeros(0, np.int64)
    first = np.r_[True, ds[1:] != ds[:-1]]
    idx_first = np.maximum.accumulate(np.where(first, np.arange(n), 0))
    rank = np.arange(n) - idx_first
    order2 = np.argsort(rank, kind="stable")
    perm = order[order2]
    return ss[perm], dd[perm], rank[order2]


def _preprocess(inputs):
    feats = np.asarray(inputs["feats"], np.float32)
    edges = [
        (np.asarray(inputs["src0"]), np.asarray(inputs["dst0"])),
        (np.asarray(inputs["src1"]), np.asarray(inputs["dst1"])),
    ]

    buckets = {}
    slice_cnt = {}   # (q,r,b) -> per-rank counts
    for q in range(T):
        src, dst = edges[q]
        for r in range(P):
            m = (dst >= r * NQ) & (dst < (r + 1) * NQ)
            es, ed = src[m], dst[m] - r * NQ
            for b in range(P):
                sel = (es >= b * NQ) & (es < (b + 1) * NQ)
                ss = (es[sel] - b * NQ).astype(np.int64)
                dd = ed[sel].astype(np.int64)
                ss, dd, rank = _rank_sort(ss, dd)
                buckets[(q, r, b)] = (ss, dd, rank)
                slice_cnt[(q, r, b)] = np.bincount(rank) if len(rank) else \
                    np.zeros(1, np.int64)

    # shared chunk plan: per bucket, rank-slice sizes = max over all cores,
    # padded to 128; slices chopped into chunks of <= CE tokens.
    chunk_plan = []         # list of (bucket, n_tokens)
    slice_max = {}          # b -> padded per-rank sizes
    for b in range(P):
        nr = max(len(slice_cnt[(q, r, b)]) for q in range(T) for r in range(P))
        sm = np.zeros(nr, np.int64)
        for q in range(T):
            for r in range(P):
                cc = slice_cnt[(q, r, b)]
                sm[: len(cc)] = np.maximum(sm[: len(cc)], cc)
        sm = ((sm + 127) // 128) * 128
        slice_max[b] = sm
        for srank in sm:
            left = int(srank)
            while left > 0:
                take = min(left, CE)
                chunk_plan.append((b, take))
                left -= take
    nch = len(chunk_plan)

    feats_bf = _bf(feats)

    in_maps = []
    for c in range(8):
        q, r = c // P, c % P
        sidx = np.zeros((nch, 128, CE // 16), np.int16)
        didx = np.zeros((nch, 128, CE // 16), np.int16)
        # build each bucket's padded token stream, then chop by chunk_plan
        streams = {}
        for b in range(P):
            ss, dd, rank = buckets[(q, r, b)]
            sm = slice_max[b]
            tot = int(sm.sum())
            sp = np.zeros(tot, np.int64)
            dp = np.full(tot, NQ, np.int64)      # pad -> dump row
            off = 0
            pos = 0
            for rr, srank in enumerate(sm):
                ncr = int(np.sum(rank == rr))
                sp[off:off + ncr] = ss[pos:pos + ncr]
                dp[off:off + ncr] = dd[pos:pos + ncr]
                pos += ncr
                off += int(srank)
            streams[b] = (sp, dp)
        cursor = {b: 0 for b in range(P)}
        for k, (b, ntok) in enumerate(chunk_plan):
            sp, dp = streams[b]
            cu = cursor[b]
            spc = np.zeros(CE, np.int64)
            dpc = np.full(CE, NQ, np.int64)
            spc[:ntok] = sp[cu:cu + ntok]
            dpc[:ntok] = dp[cu:cu + ntok]
            cursor[b] = cu + ntok
            sidx[k] = _wrap_idx(spc)
            didx[k] = _wrap_idx(dpc)

        feats_loc = np.zeros((NCP, IN), np.float32)
        feats_loc[:NQ] = feats[r * NQ:(r + 1) * NQ]

        def gat_wx(Wt, al, ar):
            Wr = Wt.reshape(Wt.shape[0], H, D)
            wal = np.einsum("khd,hd->kh", Wr, al)
            war = np.einsum("khd,hd->kh", Wr, ar)
            wx = np.concatenate([Wt, wal, war], 1)          # [F_in, 264]
            kc = wx.shape[0] // 128
            return _bf(np.ascontiguousarray(
                wx.reshape(kc, 128, 264).transpose(1, 0, 2)))

        def wchunks(Wt):
            kc = Wt.shape[0] // 128
            return _bf(np.ascontiguousarray(
                Wt.reshape(kc, 128, Wt.shape[1]).transpose(1, 0, 2)))

        def fvec(v):
            # [256] feature vector -> [128, 2, 1]  (feature = half*128 + p)
            return np.ascontiguousarray(
                np.asarray(v, np.float32).reshape(2, 128)
                .transpose(1, 0)[:, :, None])

        g = lambda k: np.asarray(inputs[k], np.float32)

        m = {
            "feats_g": feats_bf,
            "feats_loc": _bf(feats_loc),
            "sidx": sidx,
            "didx": didx,
            "w0x": gat_wx(g("gat0_W")[q], g("gat0_al")[q], g("gat0_ar")[q]),
            "w1x": gat_wx(g("gat1_W")[q], g("gat1_al")[q], g("gat1_ar")[q]),
            "b0": np.tile(g("gat0_b")[q][None, :], (128, 1)).astype(np.float32),
            "b1": np.tile(g("gat1_b")[q][None, :], (128, 1)).astype(np.float32),
            "g0w1": wchunks(g("gin0_W1")[q]),
            "g0w2": wchunks(g("gin0_W2")[q]),
            "g0b1": fvec(g("gin0_b1")[q]),
            "g0g1": fvec(g("gin0_g1")[q]),
            "g0be1": fvec(g("gin0_be1")[q]),
            "g0b2": fvec(g("gin0_b2")[q]),
            "g1w1": wchunks(g("gin1_W1")[q]),
            "g1w2": wchunks(g("gin1_W2")[q]),
            "g1b1": fvec(g("gin1_b1")[q]),
            "g1g1": fvec(g("gin1_g1")[q]),
            "g1be1": fvec(g("gin1_be1")[q]),
            "g1b2": fvec(g("gin1_b2")[q]),
            "eps0": np.full((128, 1), 1.0 + float(g("gin0_eps")[q]), np.float32),
            "eps1": np.full((128, 1), 1.0 + float(g("gin1_eps")[q]), np.float32),
            "identity": _bf(np.eye(128)),
        }
        in_maps.append(m)
    return in_maps, tuple(chunk_plan)


def _rows(dram, r0, nt, width):
    """rows [r0*128, (r0+nt)*128) of a [*, width] DRAM tensor as [128, nt, w]."""
    return dram[r0 * 128:(r0 + nt) * 128, :].rearrange("(t p) f -> p t f", p=128)


def build_program(chunk_plan):
    nc = bacc.Bacc("TRN2", target_bir_lowering=False, debug=False,
                   num_devices=8)

    dp = nc.declare_dram_parameter
    feats_g = dp("feats_g", [N, IN], BF16, isOutput=False)
    feats_loc = dp("feats_loc", [NCP, IN], BF16, isOutput=False)
    nch = len(chunk_plan)
    sidx_d = dp("sidx", [nch, 128, CE // 16], I16, isOutput=False)
    didx_d = dp("didx", [nch, 128, CE // 16], I16, isOutput=False)
    w0x_d = dp("w0x", [128, 1, 264], BF16, isOutput=False)
    w1x_d = dp("w1x", [128, 2, 264], BF16, isOutput=False)
    b0_d = dp("b0", [128, HID], FP32, isOutput=False)
    b1_d = dp("b1", [128, HID], FP32, isOutput=False)
    g0w1_d = dp("g0w1", [128, 3, HID], BF16, isOutput=False)
    g0w2_d = dp("g0w2", [128, 2, HID], BF16, isOutput=False)
    g1w1_d = dp("g1w1", [128, 2, HID], BF16, isOutput=False)
    g1w2_d = dp("g1w2", [128, 2, HID], BF16, isOutput=False)
    vec_d = {}
    for nm in ("g0b1", "g0g1", "g0be1", "g0b2",
               "g1b1", "g1g1", "g1be1", "g1b2"):
        vec_d[nm] = dp(nm, [128, 2, 1], FP32, isOutput=False)
    eps0_d = dp("eps0", [128, 1], FP32, isOutput=False)
    eps1_d = dp("eps1", [128, 1], FP32, isOutput=False)
    ident_d = dp("identity", [128, 128], BF16, isOutput=False)

    out_d = dp("out", [NCP, HID], FP32, isOutput=True)
    # scatter accumulators; ExternalOutputs are pre-zeroed by the runtime
    accg = [dp("accg0", [NCP, ACC_W], FP32, isOutput=True),
            dp("accg1", [NCP, ACC_W], FP32, isOutput=True)]
    accn = [dp("accn0", [NCP, HID + IN], BF16, isOutput=True),
            dp("accn1", [NCP, HID], BF16, isOutput=True)]

    # DRAM scratch
    zel_loc = nc.dram_tensor("zel_loc", [NCP, ZW], BF16)
    zel_full = nc.dram_tensor("zel_full", [P * NCP, ZW], BF16)
    hq_loc = nc.dram_tensor("hq_loc", [NCP, HID], BF16)
    hq_full = nc.dram_tensor("hq_full", [P * NCP, HID], BF16)
    arb_in = [nc.dram_tensor(f"arb_in{i}", [128, 4], FP32) for i in range(2)]
    arb_out = [nc.dram_tensor(f"arb_out{i}", [128, 4], FP32) for i in range(2)]
    if DEBUG_TAPS:
        dbg_xcat = nc.dram_tensor("dbg_xcat", [NCP, HID + IN], BF16)
        dbg_x1T = nc.dram_tensor("dbg_x1T", [128, 2 * NCP], BF16)
        dbg_stats = nc.dram_tensor("dbg_stats", [128, 16], FP32)

    with tile.TileContext(nc) as tc:
        cst = tc.alloc_tile_pool(name="cst", bufs=1)

        def ld(dram, shape, dtype):
            t = cst.tile(shape, dtype, tag=dram.name + "_sb")
            nc.sync.dma_start(out=t[:], in_=dram[tuple(slice(None) for _ in shape)])
            return t

        ident = ld(ident_d, [128, 128], BF16)
        w0x = ld(w0x_d, [128, 1, 264], BF16)
        w1x = ld(w1x_d, [128, 2, 264], BF16)
        b0 = ld(b0_d, [128, HID], FP32)
        b1 = ld(b1_d, [128, HID], FP32)
        g0w1 = ld(g0w1_d, [128, 3, HID], BF16)
        g0w2 = ld(g0w2_d, [128, 2, HID], BF16)
        g1w1 = ld(g1w1_d, [128, 2, HID], BF16)
        g1w2 = ld(g1w2_d, [128, 2, HID], BF16)
        vec = {nm: ld(d, [128, 2, 1], FP32) for nm, d in vec_d.items()}
        eps0 = ld(eps0_d, [128, 1], FP32)
        eps1 = ld(eps1_d, [128, 1], FP32)

        # ---------------- GAT node phase ----------------
        def gat_node(src_dram, f_in, wx):
            kc = f_in // 128
            with tc.tile_pool(name="gn", bufs=3) as pool, \
                 tc.tile_pool(name="gnp", bufs=2, space="PSUM") as pp:
                for c0 in range(0, NT, 4):
                    nt = min(4, NT - c0)
                    hsrc = pool.tile([128, nt, f_in], BF16, tag="hsrc")
                    nc.sync.dma_start(out=hsrc[:], in_=_rows(src_dram, c0, nt, f_in))
                    hT = pool.tile([128, kc, nt, 128], BF16, tag="hT")
                    for t in range(nt):
                        for k2 in range(kc):
                            pt = pp.tile([128, 128], BF16, tag="tp")
                            nc.tensor.transpose(
                                out=pt[:], in_=hsrc[:, t, k2 * 128:(k2 + 1) * 128],
                                identity=ident[:])
                            nc.any.tensor_copy(out=hT[:, k2, t, :], in_=pt[:])
                    zel = pool.tile([128, nt, ZW], BF16, tag="zel")
                    nc.vector.memset(zel[:, :, 264:ZW], 0.0)
                    for t in range(nt):
                        zp = pp.tile([128, 264], FP32, tag="zp")
                        for k2 in range(kc):
                            nc.tensor.matmul(
                                zp[:], lhsT=hT[:, k2, t, :], rhs=wx[:, k2, :],
                                start=(k2 == 0), stop=(k2 == kc - 1))
                        nc.any.tensor_copy(out=zel[:, t, 0:264], in_=zp[:])
                    nc.sync.dma_start(out=_rows(zel_loc, c0, nt, ZW), in_=zel[:])

        # ---------------- GAT edge phase ----------------
        def gat_edge(acc):
            with tc.tile_pool(name="ge", bufs=2) as pool:
                for ci, (b, ntok) in enumerate(chunk_plan):
                    if EDGE_DBG >= 4 and b != 0:
                        continue
                    sl = ntok // 128
                    st = pool.tile([128, CE // 16], I16, tag="st")
                    nc.sync.dma_start(out=st[:], in_=sidx_d[ci, :, :])
                    dt_ = pool.tile([128, CE // 16], I16, tag="dt")
                    nc.sync.dma_start(out=dt_[:], in_=didx_d[ci, :, :])
                    zg = pool.tile([128, SLOTS, ZW], BF16, tag="zg")
                    nc.gpsimd.dma_gather(
                        zg[:, 0:sl, :], zel_full[b * NCP:(b + 1) * NCP, :],
                        st[:, 0:ntok // 16], ntok, ntok, ZW)
                    if EDGE_DBG >= 3:
                        continue
                    lg = pool.tile([128, SLOTS, H], FP32, tag="lg")
                    if EDGE_DBG < 2:
                        eg = pool.tile([128, SLOTS, ZW], BF16, tag="eg")
                        nc.gpsimd.dma_gather(
                            eg[:, 0:sl, :], zel_loc[:, :],
                            dt_[:, 0:ntok // 16], ntok, ntok, ZW)
                        nc.vector.tensor_tensor(
                            out=lg[:, 0:sl, :], in0=zg[:, 0:sl, 256:260],
                            in1=eg[:, 0:sl, 260:264], op=ALU.add)
                    else:
                        nc.vector.tensor_copy(out=lg[:, 0:sl, :],
                                              in_=zg[:, 0:sl, 256:260])
                    lr = pool.tile([128, SLOTS, H], FP32, tag="lr")
                    nc.vector.scalar_tensor_tensor(
                        out=lr[:, 0:sl, :], in0=lg[:, 0:sl, :], scalar=0.2,
                        in1=lg[:, 0:sl, :], op0=ALU.mult, op1=ALU.max)
                    w_ = pool.tile([128, SLOTS, H], FP32, tag="w")
                    nc.scalar.activation(out=w_[:, 0:sl, :],
                                         in_=lr[:, 0:sl, :], func=AF.Exp)
                    msg = pool.tile([128, SLOTS, ACC_W], FP32, tag="msg")
                    nc.vector.memset(msg[:, 0:sl, 260:ACC_W], 0.0)
                    nc.vector.tensor_tensor(
                        out=msg[:, 0:sl, 0:256].rearrange(
                            "p s (h d) -> p s h d", h=H),
                        in0=zg[:, 0:sl, 0:256].rearrange(
                            "p s (h d) -> p s h d", h=H),
                        in1=w_[:, 0:sl, :].unsqueeze(3).broadcast_to(
                            [128, sl, H, D]),
                        op=ALU.mult)
                    nc.vector.tensor_copy(out=msg[:, 0:sl, 256:260],
                                          in_=w_[:, 0:sl, :])
                    if EDGE_DBG < 1:
                        nc.gpsimd.dma_scatter_add(
                            acc[:, :], msg[:, 0:sl, :], dt_[:, 0:ntok // 16],
                            ntok, ntok, ACC_W)

        # ---------------- GAT post (normalize + bias + relu) ----------------
        def gat_post(acc, bias, dst_dram):
            with tc.tile_pool(name="gp", bufs=3) as pool:
                for c0 in range(0, NT, 4):
                    nt = min(4, NT - c0)
                    a = pool.tile([128, nt, 260], FP32, tag="ac")
                    nc.sync.dma_start(
                        out=a[:],
                        in_=acc[c0 * 128:(c0 + nt) * 128, 0:260].rearrange(
                            "(t p) f -> p t f", p=128))
                    dmax = pool.tile([128, nt, H], FP32, tag="dmax")
                    nc.vector.tensor_scalar_max(dmax[:], a[:, :, 256:260], 1e-9)
                    rec = pool.tile([128, nt, H], FP32, tag="rec")
                    nc.vector.reciprocal(rec[:], dmax[:])
                    hb = pool.tile([128, nt, HID], FP32, tag="hb")
                    nc.vector.tensor_tensor(
                        out=hb[:].rearrange("p s (h d) -> p s h d", h=H),
                        in0=a[:, :, 0:256].rearrange("p s (h d) -> p s h d", h=H),
                        in1=rec[:].unsqueeze(3).broadcast_to([128, nt, H, D]),
                        op=ALU.mult)
                    hb2 = pool.tile([128, nt, HID], FP32, tag="hb2")
                    nc.vector.tensor_tensor(
                        out=hb2[:], in0=hb[:],
                        in1=bias[:].unsqueeze(1).broadcast_to([128, nt, HID]),
                        op=ALU.add)
                    ht = pool.tile([128, nt, HID], BF16, tag="ht")
                    nc.scalar.activation(out=ht[:], in_=hb2[:], func=AF.Relu)
                    nc.sync.dma_start(out=_rows(dst_dram, c0, nt, HID), in_=ht[:])

        # ---------------- GIN edge phase ----------------
        def gin_edge(gin):
            acc = accn[gin]
            step = (HID + IN) if gin == 0 else HID
            with tc.tile_pool(name="ne", bufs=2) as pool:
                for ci, (b, ntok) in enumerate(chunk_plan):
                    if EDGE_DBG >= 4 and b != 0:
                        continue
                    sl = ntok // 128
                    st = pool.tile([128, CE // 16], I16, tag="st")
                    nc.sync.dma_start(out=st[:], in_=sidx_d[ci, :, :])
                    dt_ = pool.tile([128, CE // 16], I16, tag="dt")
                    nc.sync.dma_start(out=dt_[:], in_=didx_d[ci, :, :])
                    hg = pool.tile([128, SLOTS, HID], BF16, tag="hg")
                    nc.gpsimd.dma_gather(
                        hg[:, 0:sl, :], hq_full[b * NCP:(b + 1) * NCP, :],
                        st[:, 0:ntok // 16], ntok, ntok, HID)
                    nc.gpsimd.dma_scatter_add(
                        acc[:, 0:HID], hg[:, 0:sl, :], dt_[:, 0:ntok // 16],
                        ntok, ntok, HID, elem_step=step)
                    if gin == 0:
                        fg = pool.tile([128, SLOTS, IN], BF16, tag="fg")
                        nc.gpsimd.dma_gather(
                            fg[:, 0:sl, :], feats_g[b * NQ:(b + 1) * NQ, :],
                            st[:, 0:ntok // 16], ntok, ntok, IN)
                        nc.gpsimd.dma_scatter_add(
                            acc[:, HID:HID + IN], fg[:, 0:sl, :],
                            dt_[:, 0:ntok // 16], ntok, ntok, IN,
                            elem_step=step)

        # ---------------- GIN node phase ----------------
        def gin_node(gin, dst_dram, out_f32):
            acc = accn[gin]
            w_in = (HID + IN) if gin == 0 else HID
            kc = w_in // 128
            w1 = g0w1 if gin == 0 else g1w1
            w2 = g0w2 if gin == 0 else g1w2
            epsv = eps0 if gin == 0 else eps1
            pre = "g0" if gin == 0 else "g1"
            with tc.tile_pool(name="nn", bufs=3) as pool, \
                 tc.tile_pool(name="nnb", bufs=1) as big, \
                 tc.tile_pool(name="nnp", bufs=2, space="PSUM") as pp:
                x1T = big.tile([128, 2, NCP], BF16, tag="x1T")
                run_s = big.tile([128, 2, 1], FP32, tag="run_s")
                run_q = big.tile([128, 2, 1], FP32, tag="run_q")
                nc.vector.memset(run_s[:], 0.0)
                nc.vector.memset(run_q[:], 0.0)
                # pass A: x1^T = W1^T @ xcat^T (feature-major), plus stats
                for c0 in range(0, NT, 4):
                    nt = min(4, NT - c0)
                    a = pool.tile([128, nt, w_in], BF16, tag="a")
                    nc.sync.dma_start(out=a[:], in_=_rows(acc, c0, nt, w_in))
                    hs = pool.tile([128, nt, HID], BF16, tag="hs")
                    nc.sync.dma_start(out=hs[:], in_=_rows(hq_loc, c0, nt, HID))
                    xc = pool.tile([128, nt, w_in], BF16, tag="xc")
                    nc.vector.scalar_tensor_tensor(
                        out=xc[:, :, 0:HID], in0=hs[:], scalar=epsv[:],
                        in1=a[:, :, 0:HID], op0=ALU.mult, op1=ALU.add)
                    if gin == 0:
                        fs = pool.tile([128, nt, IN], BF16, tag="fs")
                        nc.sync.dma_start(out=fs[:],
                                          in_=_rows(feats_loc, c0, nt, IN))
                        nc.vector.scalar_tensor_tensor(
                            out=xc[:, :, HID:w_in], in0=fs[:], scalar=epsv[:],
                            in1=a[:, :, HID:w_in], op0=ALU.mult, op1=ALU.add)
                    if DEBUG_TAPS and gin == 0:
                        nc.sync.dma_start(
                            out=_rows(dbg_xcat, c0, nt, w_in), in_=xc[:])
                    xT = pool.tile([128, kc, nt, 128], BF16, tag="xT")
                    for t in range(nt):
                        for k2 in range(kc):
                            pt = pp.tile([128, 128], BF16, tag="tp2")
                            nc.tensor.transpose(
                                out=pt[:], in_=xc[:, t, k2 * 128:(k2 + 1) * 128],
                                identity=ident[:])
                            nc.any.tensor_copy(out=xT[:, k2, t, :], in_=pt[:])
                    for hf in range(2):
                        xp = pp.tile([128, 512], FP32, tag="x1p")
                        for k2 in range(kc):
                            nc.tensor.matmul(
                                xp[:, 0:nt * 128],
                                lhsT=w1[:, k2, hf * 128:(hf + 1) * 128],
                                rhs=xT[:, k2, :, :].rearrange(
                                    "p t f -> p (t f)"),
                                start=(k2 == 0), stop=(k2 == kc - 1))
                        # stats over REAL nodes only (exclude pad/dump rows)
                        real = min(nt * 128, max(0, NQ - c0 * 128))
                        if real > 0:
                            sq = pool.tile([128, 512], BF16, tag="sq")
                            sqa = pool.tile([128, 1], FP32, tag="sqa")
                            nc.scalar.activation(
                                out=sq[:, 0:real], in_=xp[:, 0:real],
                                func=AF.Square, accum_out=sqa[:])
                            sm = pool.tile([128, 1], FP32, tag="sm")
                            nc.vector.tensor_reduce(
                                out=sm[:], in_=xp[:, 0:real],
                                axis=mybir.AxisListType.X, op=ALU.add)
                            nc.vector.tensor_add(run_q[:, hf, :],
                                                 run_q[:, hf, :], sqa[:])
                            nc.vector.tensor_add(run_s[:, hf, :],
                                                 run_s[:, hf, :], sm[:])
                        nc.vector.tensor_copy(
                            out=x1T[:, hf, c0 * 128:(c0 + nt) * 128],
                            in_=xp[:, 0:nt * 128])
                if DEBUG_TAPS and gin == 0:
                    nc.sync.dma_start(
                        out=dbg_x1T[:, :],
                        in_=x1T[:].rearrange("p a b -> p (a b)"))
                # stats allreduce
                arp = pool.tile([128, 4], FP32, tag="arp")
                nc.vector.tensor_copy(out=arp[:, 0:2], in_=run_s[:, :, 0])
                nc.vector.tensor_copy(out=arp[:, 2:4], in_=run_q[:, :, 0])
                nc.sync.dma_start(out=arb_in[gin][:, :], in_=arp[:])
                nc.gpsimd.collective_compute(
                    "AllReduce", ALU.add, replica_groups=RGROUPS,
                    ins=[arb_in[gin][:, :].opt()],
                    outs=[arb_out[gin][:, :].opt()])
                art = pool.tile([128, 4], FP32, tag="art")
                nc.sync.dma_start(out=art[:], in_=arb_out[gin][:, :])
                mu = pool.tile([128, 2], FP32, tag="mu")
                nc.vector.tensor_scalar_mul(mu[:], art[:, 0:2], 1.0 / N)
                msq = pool.tile([128, 2], FP32, tag="msq")
                nc.vector.tensor_scalar_mul(msq[:], art[:, 2:4], 1.0 / N)
                mu2 = pool.tile([128, 2], FP32, tag="mu2")
                nc.vector.tensor_mul(mu2[:], mu[:], mu[:])
                var = pool.tile([128, 2], FP32, tag="var")
                nc.vector.tensor_sub(var[:], msq[:], mu2[:])
                vare = pool.tile([128, 2], FP32, tag="vare")
                nc.vector.tensor_scalar_add(vare[:], var[:], BN_EPS)
                sd = pool.tile([128, 2], FP32, tag="sd")
                nc.scalar.activation(out=sd[:], in_=vare[:], func=AF.Sqrt)
                rsd = pool.tile([128, 2], FP32, tag="rsd")
                nc.vector.reciprocal(rsd[:], sd[:])
                # bn: (x1 + b1 - (mu1 + b1)) * scale + be  -- b1 cancels
                scl = pool.tile([128, 2], FP32, tag="scl")
                nc.vector.tensor_mul(scl[:], rsd[:], vec[pre + "g1"][:, :, 0])
                mus = pool.tile([128, 2], FP32, tag="mus")
                nc.vector.tensor_mul(mus[:], mu[:], scl[:])
                shf = pool.tile([128, 2], FP32, tag="shf")
                nc.vector.tensor_sub(shf[:], vec[pre + "be1"][:, :, 0], mus[:])
                if DEBUG_TAPS and gin == 0:
                    dst_ = pool.tile([128, 16], FP32, tag="dbgst")
                    nc.vector.tensor_copy(dst_[:, 0:2], run_s[:, :, 0])
                    nc.vector.tensor_copy(dst_[:, 2:4], run_q[:, :, 0])
                    nc.vector.tensor_copy(dst_[:, 4:6], mu[:])
                    nc.vector.tensor_copy(dst_[:, 6:8], var[:])
                    nc.vector.tensor_copy(dst_[:, 8:10], scl[:])
                    nc.vector.tensor_copy(dst_[:, 10:12], shf[:])
                    nc.vector.tensor_copy(dst_[:, 12:14], art[:, 0:2])
                    nc.vector.tensor_copy(dst_[:, 14:16], art[:, 2:4])
                    nc.sync.dma_start(out=dbg_stats[:, :], in_=dst_[:])
                # pass B: bn+relu, second matmul, +b2, relu, transpose out
                for c0 in range(0, NT, 4):
                    nt = min(4, NT - c0)
                    x1n = pool.tile([128, 2, 512], BF16, tag="x1n")
                    for hf in range(2):
                        nc.scalar.activation(
                            out=x1n[:, hf, 0:nt * 128],
                            in_=x1T[:, hf, c0 * 128:(c0 + nt) * 128],
                            func=AF.Relu, scale=scl[:, hf:hf + 1],
                            bias=shf[:, hf:hf + 1])
                    ho = pool.tile([128, 2, 512], BF16, tag="ho")
                    for hf in range(2):
                        x2p = pp.tile([128, 512], FP32, tag="x2p")
                        for k2 in range(2):
                            nc.tensor.matmul(
                                x2p[:, 0:nt * 128],
                                lhsT=w2[:, k2, hf * 128:(hf + 1) * 128],
                                rhs=x1n[:, k2, 0:nt * 128],
                                start=(k2 == 0), stop=(k2 == 1))
                        nc.scalar.activation(
                            out=ho[:, hf, 0:nt * 128], in_=x2p[:, 0:nt * 128],
                            func=AF.Relu, bias=vec[pre + "b2"][:, hf, :])
                    hout = pool.tile([128, nt, HID],
                                     FP32 if out_f32 else BF16, tag="hout")
                    for t in range(nt):
                        for hf in range(2):
                            pt = pp.tile([128, 128], BF16, tag="tp2")
                            nc.tensor.transpose(
                                out=pt[:],
                                in_=ho[:, hf, t * 128:(t + 1) * 128],
                                identity=ident[:])
                            nc.any.tensor_copy(
                                out=hout[:, t, hf * 128:(hf + 1) * 128],
                                in_=pt[:])
                    nc.sync.dma_start(out=_rows(dst_dram, c0, nt, HID),
                                      in_=hout[:])

        def allgather(src, dstf):
            nc.gpsimd.collective_compute(
                "AllGather", ALU.bypass, replica_groups=RGROUPS,
                ins=[src[:, :].opt()], outs=[dstf[:, :].opt()])

        # ---------------- zero the scatter accumulators ----------------
        with tc.tile_pool(name="zz", bufs=1) as zp:
            for acc_t, wdt, dt_ in ((accg[0], ACC_W, FP32),
                                    (accg[1], ACC_W, FP32),
                                    (accn[0], HID + IN, BF16),
                                    (accn[1], HID, BF16)):
                zt = zp.tile([128, 8, wdt], dt_, tag="z_" + acc_t.name)
                nc.vector.memset(zt[:], 0.0)
                for r0 in range(0, NT, 8):
                    nt = min(8, NT - r0)
                    nc.sync.dma_start(out=_rows(acc_t, r0, nt, wdt),
                                      in_=zt[:, 0:nt, :])

        # ---------------- full schedule ----------------
        gat_node(feats_loc, IN, w0x)
        if STAGES >= 2:
            allgather(zel_loc, zel_full)
        if STAGES >= 3:
            gat_edge(accg[0])
        if STAGES >= 4:
            gat_post(accg[0], b0, hq_loc)
        if STAGES >= 5:
            gat_node(hq_loc, HID, w1x)
            allgather(zel_loc, zel_full)
            gat_edge(accg[1])
            gat_post(accg[1], b1, hq_loc)
        if STAGES >= 6:
            allgather(hq_loc, hq_full)
            gin_edge(0)
            gin_node(0, hq_loc, out_f32=False)
        if STAGES >= 7:
            allgather(hq_loc, hq_full)
            gin_edge(1)
            gin_node(1, out_d, out_f32=True)

        cst.release()

    nc.compile()
    return nc


_CACHE = {}


def kernel(**inputs):
    in_maps, chunk_plan = _preprocess(inputs)
    nc = _CACHE.get(chunk_plan)
    if nc is None:
        nc = build_program(chunk_plan)
        _CACHE[chunk_plan] = nc
    res = run_bass_kernel_spmd(nc, in_maps, core_ids=list(range(8)))
    out = np.zeros((N, T * HID), np.float32)
    for c in range(8):
        q, r = c // P, c % P
        out[r * NQ:(r + 1) * NQ, q * HID:(q + 1) * HID] = \
            np.asarray(res.results[c]["out"], np.float32)[:NQ]
    return out



# revision 14
# speedup vs baseline: 2.1656x; 2.1656x over previous
"""GNN message-passing (2x GAT + 2x GIN, 2 edge types) on 8 trn2 NeuronCores.

v2 design — scatter-free, SWDGE-minimal:

Sharding: cores 0-3 handle edge type 0, cores 4-7 type 1. Within a quad,
nodes are sharded by dst range (12500/core, padded to 12544). Edges live on
the core owning their dst, sorted by 128-node dst block.

Per edge phase, per group of 4 dst blocks: ONE dma_gather per source
half-slice (2 total; half-slice tensors of 25088 rows keep indices int16)
fetches packed src rows token-major; a one-hot selection matrix SE[e,d] =
(dstv[e] == iota[d]) built by a single DVE compare turns the per-dst-block
aggregation into PE matmuls accumulating in PSUM (no dma_scatter_add at
all). GAT's per-edge er[dst] comes from a third gather over a replicated-row
er table (256B rows). Edge softmax needs no segment-max (logits are O(1)).

GAT0's projections are computed redundantly for ALL nodes from the
replicated feats input, so layer 0 needs no AllGather. Later AllGathers
(zel1, hcat, h3) run per row-half on half-split tensors to overlap with
compute. GIN BatchNorm stats are per-feature PSUM accumulators (ones-vector
matmuls) reduced by a tiny quad AllReduce; b1 cancels in the BN shift.
"""

import sys

for _p in ("/opt/trn_rl_repo",):
    if _p not in sys.path:
        sys.path.insert(0, _p)

import numpy as np
import ml_dtypes

import concourse.bacc as bacc
import concourse.bass as bass
import concourse.tile as tile
import concourse.mybir as mybir
from concourse.bass_utils import run_bass_kernel_spmd

FP32 = mybir.dt.float32
BF16 = mybir.dt.bfloat16
I16 = mybir.dt.int16
AF = mybir.ActivationFunctionType
ALU = mybir.AluOpType

# problem constants
N, IN, HID, H, D = 50000, 128, 256, 4, 64
E, T = 400000, 2
BN_EPS = 1e-5
P = 4                     # cores per quad
NQ = 12500                # real nodes per core
NCP = 12544               # padded (98 * 128)
HS = NCP // 2             # 6272 rows per half of a core's range
SR = P * HS               # 25088 rows per half-slice tensor
NB = NCP // 128           # 98 dst blocks
HB = NB // 2              # 49 blocks per half
GB = 4                    # dst blocks per gather group
ZW = 384                  # packed row: [z 256 | el 4 | er 4 | pad]
ERW = 128                 # replicated er row (bf16 -> 256B)
PADV = 300                # dstv pad marker (outside 0..127)
RGROUPS = [[0, 1, 2, 3], [4, 5, 6, 7]]
import os
STAGES = int(os.environ.get("GNN_STAGES", "99"))


def _bf(x):
    return np.asarray(x, dtype=ml_dtypes.bfloat16)


def _wrap_idx(a):
    """[n] ints (n % 16 == 0) -> [128, n//16] int16 SWDGE wrapped layout
    (token i at [i % 16, i // 16], replicated across the 8 Q7 cores)."""
    w = a.reshape(-1, 16).T.astype(np.int16)
    return np.tile(w, (8, 1))


def _tok_major(a):
    """[n] values (n % 128 == 0) -> [128, n//128] token-major."""
    return a.reshape(-1, 128).T


def _preprocess(inputs):
    feats = np.asarray(inputs["feats"], np.float32)
    edges = [
        (np.asarray(inputs["src0"]), np.asarray(inputs["dst0"])),
        (np.asarray(inputs["src1"]), np.asarray(inputs["dst1"])),
    ]

    # ---- edge buckets per core / dst block / src half-slice ----
    per_core = []
    for q in range(T):
        src, dst = edges[q]
        for r in range(P):
            m = (dst >= r * NQ) & (dst < (r + 1) * NQ)
            g = src[m].astype(np.int64)
            j = (dst[m] - r * NQ).astype(np.int64)
            rs = g // NQ
            is_ = g - rs * NQ
            s = is_ // HS
            row = rs * HS + (is_ - s * HS)      # row in half-slice tensor
            blk = j // 128
            buckets = {}
            for b in range(NB):
                mb_ = blk == b
                for sl in range(2):
                    sel = mb_ & (s == sl)
                    buckets[(b, sl)] = (row[sel], j[sel])
            per_core.append(buckets)

    # shared plan: per (block, slice) padded counts = max over 8 cores
    nbs = np.zeros((NB, 2), np.int64)
    for b in range(NB):
        for sl in range(2):
            mx = max(len(per_core[c][(b, sl)][0]) for c in range(8))
            nbs[b, sl] = ((mx + 127) // 128) * 128

    groups = []
    for g0 in range(0, NB, GB):
        blocks = tuple(range(g0, min(g0 + GB, NB)))
        k0 = int(sum(nbs[b, 0] for b in blocks)) // 128
        k1 = int(sum(nbs[b, 1] for b in blocks)) // 128
        slotmap = []
        for sl in range(2):
            for bi, b in enumerate(blocks):
                slotmap += [(bi, sl)] * (int(nbs[b, sl]) // 128)
        groups.append((blocks, k0, k1, tuple(slotmap)))
    plan_key = tuple(groups)

    ip_cols = []
    for (blocks, k0, k1, _) in groups:
        n0, n1 = k0 * 128, k1 * 128
        ip_cols.append(n0 // 16 + n1 // 16)
    IPW = int(np.sum(ip_cols))
    DVW = int(sum(k0 + k1 for (_, k0, k1, _) in groups))

    fpad = np.zeros((P, NCP, IN), np.float32)
    for rr in range(P):
        fpad[rr, :NQ] = feats[rr * NQ:(rr + 1) * NQ]
    feats_s = np.zeros((2, SR, IN), np.float32)
    for rr in range(P):
        feats_s[0, rr * HS:(rr + 1) * HS] = fpad[rr, 0:HS]
        feats_s[1, rr * HS:(rr + 1) * HS] = fpad[rr, HS:2 * HS]

    in_maps = []
    for c in range(8):
        q, r = c // P, c % P
        buckets = per_core[c]
        ip = np.zeros((128, IPW), np.int16)
        dv = np.zeros((128, DVW), np.float32)
        ipo = 0
        dvo = 0
        for gi, (blocks, k0, k1, _) in enumerate(groups):
            zi = [[], []]
            dvv = []
            for sl in range(2):
                for b in blocks:
                    rows, js = buckets[(b, sl)]
                    n = int(nbs[b, sl])
                    rpad = np.zeros(n, np.int64)
                    dpad = np.full(n, PADV, np.int64)
                    rpad[: len(rows)] = rows
                    dpad[: len(js)] = js - b * 128
                    zi[sl].append(rpad)
                    dvv.append(dpad)
            z0 = (np.concatenate(zi[0]) if zi[0] else np.zeros(0, np.int64))
            z1 = (np.concatenate(zi[1]) if zi[1] else np.zeros(0, np.int64))
            dvs = np.concatenate(dvv)
            for arr in (z0, z1):
                if len(arr):
                    w = _wrap_idx(arr)
                    ip[:, ipo:ipo + w.shape[1]] = w
                    ipo += w.shape[1]
            ns = len(dvs) // 128
            dv[:, dvo:dvo + ns] = _tok_major(dvs.astype(np.float32))
            dvo += ns
        assert ipo == IPW and dvo == DVW, (ipo, IPW, dvo, DVW)

        def gat_wx(Wt, al, ar):
            Wr = Wt.reshape(Wt.shape[0], H, D)
            wal = np.einsum("khd,hd->kh", Wr, al)
            war = np.einsum("khd,hd->kh", Wr, ar)
            wx = np.concatenate([Wt, wal, war], 1)          # [F_in, 264]
            kc = wx.shape[0] // 128
            return _bf(np.ascontiguousarray(
                wx.reshape(kc, 128, 264).transpose(1, 0, 2)))

        def wchunks(Wt):
            kc = Wt.shape[0] // 128
            return _bf(np.ascontiguousarray(
                Wt.reshape(kc, 128, Wt.shape[1]).transpose(1, 0, 2)))

        def fvec(v):
            return np.ascontiguousarray(
                np.asarray(v, np.float32).reshape(2, 128)
                .transpose(1, 0)[:, :, None])

        g = lambda k: np.asarray(inputs[k], np.float32)

        ers0 = _wrap_idx(np.arange(r * HS, (r + 1) * HS, dtype=np.int64))
        ers1 = ers0.copy()

        m = {
            "feats_a0": _bf(feats_s[0]),
            "feats_a1": _bf(feats_s[1]),
            "feats_loc": _bf(fpad[r]),
            "idxpack": ip,
            "dstv": dv.astype(ml_dtypes.bfloat16),
            "ersrc0": ers0,
            "ersrc1": ers1,
            "w0x": gat_wx(g("gat0_W")[q], g("gat0_al")[q], g("gat0_ar")[q]),
            "w1x": gat_wx(g("gat1_W")[q], g("gat1_al")[q], g("gat1_ar")[q]),
            "b0": np.tile(g("gat0_b")[q][None, :], (128, 1)).astype(np.float32),
            "b1": np.tile(g("gat1_b")[q][None, :], (128, 1)).astype(np.float32),
            "g0w1": wchunks(g("gin0_W1")[q]),
            "g0w2": wchunks(g("gin0_W2")[q]),
            "g1w1": wchunks(g("gin1_W1")[q]),
            "g1w2": wchunks(g("gin1_W2")[q]),
            "g0g1": fvec(g("gin0_g1")[q]),
            "g0be1": fvec(g("gin0_be1")[q]),
            "g1g1": fvec(g("gin1_g1")[q]),
            "g1be1": fvec(g("gin1_be1")[q]),
            "g0b2t": np.tile(g("gin0_b2")[q][None, :], (128, 1)).astype(np.float32),
            "g1b2t": np.tile(g("gin1_b2")[q][None, :], (128, 1)).astype(np.float32),
            "eps0": np.full((128, 1), 1.0 + float(g("gin0_eps")[q]), np.float32),
            "eps1": np.full((128, 1), 1.0 + float(g("gin1_eps")[q]), np.float32),
            "identity": _bf(np.eye(128)),
            "identity_f": np.eye(128, dtype=np.float32),
            "iota": _bf(np.tile(np.arange(128, dtype=np.float32)[None, :],
                                (128, 1))),
            "ones_col": _bf(np.ones((128, 1), np.float32)),
            "ones_row": np.ones((1, 128), np.float32),
            "padmask": np.concatenate([
                np.ones((NQ - (NB - 1) * 128, 1), np.float32),
                np.zeros((NCP - NQ, 1), np.float32)]),
        }
        in_maps.append(m)
    return in_maps, (plan_key, IPW, DVW)


def _rows(dram, r0, nt, width):
    return dram[r0 * 128:(r0 + nt) * 128, :].rearrange("(t p) f -> p t f", p=128)


def build_program(plan):
    plan_key, IPW, DVW = plan
    groups = list(plan_key)   # (blocks, k0, k1, slotmap)

    nc = bacc.Bacc("TRN2", target_bir_lowering=False, debug=False,
                   num_devices=8)

    dp = nc.declare_dram_parameter
    feats_a = [dp("feats_a0", [SR, IN], BF16, isOutput=False),
               dp("feats_a1", [SR, IN], BF16, isOutput=False)]
    feats_loc_d = dp("feats_loc", [NCP, IN], BF16, isOutput=False)
    ip_d = dp("idxpack", [128, IPW], I16, isOutput=False)
    dv_d = dp("dstv", [128, DVW], BF16, isOutput=False)
    ersrc_d = [dp("ersrc0", [128, HS // 16], I16, isOutput=False),
               dp("ersrc1", [128, HS // 16], I16, isOutput=False)]
    w0x_d = dp("w0x", [128, 1, 264], BF16, isOutput=False)
    w1x_d = dp("w1x", [128, 2, 264], BF16, isOutput=False)
    b0_d = dp("b0", [128, HID], FP32, isOutput=False)
    b1_d = dp("b1", [128, HID], FP32, isOutput=False)
    g0w1_d = dp("g0w1", [128, 3, HID], BF16, isOutput=False)
    g0w2_d = dp("g0w2", [128, 2, HID], BF16, isOutput=False)
    g1w1_d = dp("g1w1", [128, 2, HID], BF16, isOutput=False)
    g1w2_d = dp("g1w2", [128, 2, HID], BF16, isOutput=False)
    vec_d = {nm: dp(nm, [128, 2, 1], FP32, isOutput=False)
             for nm in ("g0g1", "g0be1", "g1g1", "g1be1")}
    b2t_d = {nm: dp(nm, [128, HID], FP32, isOutput=False)
             for nm in ("g0b2t", "g1b2t")}
    eps0_d = dp("eps0", [128, 1], FP32, isOutput=False)
    eps1_d = dp("eps1", [128, 1], FP32, isOutput=False)
    ident_d = dp("identity", [128, 128], BF16, isOutput=False)
    identf_d = dp("identity_f", [128, 128], FP32, isOutput=False)
    iota_d = dp("iota", [128, 128], BF16, isOutput=False)
    onesc_d = dp("ones_col", [128, 1], BF16, isOutput=False)
    onesr_d = dp("ones_row", [1, 128], FP32, isOutput=False)
    padmask_d = dp("padmask", [128, 1], FP32, isOutput=False)

    out_d = dp("out", [NCP, HID], FP32, isOutput=True)

    # DRAM scratch. *_loc tensors are split in row halves so each
    # AllGather half only depends on the blocks that feed it.
    zel0_s = [nc.dram_tensor(f"zel0_s{i}", [SR, ZW], BF16) for i in range(2)]
    zel1_s = [nc.dram_tensor(f"zel1_s{i}", [SR, ZW], BF16) for i in range(2)]
    hcat_s = [nc.dram_tensor(f"hcat_s{i}", [SR, ZW], BF16) for i in range(2)]
    h3_s = [nc.dram_tensor(f"h3_s{i}", [SR, HID], BF16) for i in range(2)]
    zel1_loc = [nc.dram_tensor(f"zel1_loc{i}", [HS, ZW], BF16)
                for i in range(2)]
    hcat_loc = [nc.dram_tensor(f"hcat_loc{i}", [HS, ZW], BF16)
                for i in range(2)]
    h3_loc = [nc.dram_tensor(f"h3_loc{i}", [HS, HID], BF16)
              for i in range(2)]
    er_cmp = [nc.dram_tensor(f"er_cmp{i}", [NCP, 4], BF16)
              for i in range(2)]
    arb_in = [nc.dram_tensor(f"arb_in{i}", [128, 4], FP32) for i in range(2)]
    scl_dram = [nc.dram_tensor(f"scl_dram{i}", [4, 128], FP32)
                for i in range(2)]
    arb_out = [nc.dram_tensor(f"arb_out{i}", [128, 4], FP32) for i in range(2)]

    def loc_rows(halves, b, width):
        """[128, width] AP for dst-block b of a half-split row tensor."""
        half, bb = (0, b) if b < HB else (1, b - HB)
        return halves[half][bb * 128:(bb + 1) * 128, 0:width].rearrange(
            "(t p) f -> p t f", p=128)[:, 0, :]

    ip_off, dv_off = [], []
    o1, o2 = 0, 0
    for (blocks, k0, k1, _) in groups:
        ip_off.append(o1)
        dv_off.append(o2)
        n0, n1 = k0 * 128, k1 * 128
        o1 += n0 // 16 + n1 // 16
        o2 += k0 + k1
    maxslots = max(k0 + k1 for (_, k0, k1, _) in groups)

    with tile.TileContext(nc) as tc:
        cst = tc.alloc_tile_pool(name="cst", bufs=1)

        def ld(dram, shape, dtype):
            t = cst.tile(shape, dtype, tag=dram.name + "_sb")
            nc.sync.dma_start(out=t[:],
                              in_=dram[tuple(slice(None) for _ in shape)])
            return t

        ident = ld(ident_d, [128, 128], BF16)
        identf = ld(identf_d, [128, 128], FP32)
        iota = ld(iota_d, [128, 128], BF16)
        onesc = ld(onesc_d, [128, 1], BF16)
        onesr = ld(onesr_d, [1, 128], FP32)
        padmask = ld(padmask_d, [128, 1], FP32)
        w0x = ld(w0x_d, [128, 1, 264], BF16)
        w1x = ld(w1x_d, [128, 2, 264], BF16)
        b0 = ld(b0_d, [128, HID], FP32)
        b1 = ld(b1_d, [128, HID], FP32)
        g0w1 = ld(g0w1_d, [128, 3, HID], BF16)
        g0w2 = ld(g0w2_d, [128, 2, HID], BF16)
        g1w1 = ld(g1w1_d, [128, 2, HID], BF16)
        g1w2 = ld(g1w2_d, [128, 2, HID], BF16)
        vec = {nm: ld(d, [128, 2, 1], FP32) for nm, d in vec_d.items()}
        b2t = {nm: ld(d, [128, HID], FP32) for nm, d in b2t_d.items()}
        eps0 = ld(eps0_d, [128, 1], FP32)
        eps1 = ld(eps1_d, [128, 1], FP32)

        big = tc.alloc_tile_pool(name="big", bufs=1)
        x1_sb = big.tile([128, NB, HID], BF16, tag="x1_sb")

        # ---------------- GAT0 node: all nodes, no AG ----------------
        def gat0_node():
            with tc.tile_pool(name="n0", bufs=3) as pool, \
                 tc.tile_pool(name="n0p", bufs=2, space="PSUM") as pp:
                for sl in range(2):
                    ntile = SR // 128        # 196
                    for t0 in range(0, ntile, 4):
                        nt = min(4, ntile - t0)
                        ft = pool.tile([128, 4, IN], BF16, tag="ft")
                        nc.sync.dma_start(out=ft[:, 0:nt, :],
                                          in_=_rows(feats_a[sl], t0, nt, IN))
                        zel = pool.tile([128, 4, 264], BF16, tag="zel")
                        for t in range(nt):
                            pt = pp.tile([128, 128], BF16, tag="tp")
                            nc.tensor.transpose(out=pt[:], in_=ft[:, t, :],
                                                identity=ident[:])
                            fT = pool.tile([128, 128], BF16, tag="fT")
                            nc.any.tensor_copy(out=fT[:], in_=pt[:])
                            zp = pp.tile([128, 512], FP32, tag="zp")
                            nc.tensor.matmul(zp[:, 0:264], lhsT=fT[:],
                                             rhs=w0x[:, 0, :],
                                             start=True, stop=True)
                            nc.any.tensor_copy(out=zel[:, t, :],
                                               in_=zp[:, 0:264])
                        nc.sync.dma_start(
                            out=zel0_s[sl][t0 * 128:(t0 + nt) * 128, 0:264]
                            .rearrange("(t p) f -> p t f", p=128),
                            in_=zel[:, 0:nt, :])

        # er_cmp0[i] <- zel0_s[sl][own rows, 260:264]
        def er_fill():
            with tc.tile_pool(name="ef", bufs=2) as pool:
                for sl in range(2):
                    st = pool.tile([128, HS // 16], I16, tag="efst")
                    nc.sync.dma_start(out=st[:], in_=ersrc_d[sl][:, :])
                    zg = pool.tile([128, HS // 128, ZW], BF16, tag="efzg")
                    for p0 in range(0, HS, 896):
                        nc.gpsimd.dma_gather(
                            zg[:, p0 // 128:(p0 + 896) // 128, :],
                            zel0_s[sl][:, :],
                            st[:, p0 // 16:(p0 + 896) // 16], 896, 896, ZW)
                    erb = pool.tile([128, HS // 128, 4], BF16, tag="efb")
                    nc.vector.tensor_copy(out=erb[:], in_=zg[:, :, 260:264])
                    nc.sync.dma_start(
                        out=er_cmp[0][sl * HS:(sl + 1) * HS, :].rearrange(
                            "(t p) f -> p t f", p=128),
                        in_=erb[:])

        # hcat_loc cols 256:384 <- feats_loc
        def hcat_prefill():
            with tc.tile_pool(name="hp", bufs=2) as pool:
                for half in range(2):
                    for t0 in range(0, HB, 7):
                        ftl = pool.tile([128, 7, IN], BF16, tag="ftl")
                        nc.sync.dma_start(
                            out=ftl[:],
                            in_=_rows(feats_loc_d, half * HB + t0, 7, IN))
                        nc.sync.dma_start(
                            out=hcat_loc[half][t0 * 128:(t0 + 7) * 128,
                                               256:384]
                            .rearrange("(t p) f -> p t f", p=128),
                            in_=ftl[:])

        MAXTOK = 1024   # SWDGE ring holds 1024 descriptors

        def gather_split(zg, src_ap, ipt, col0, slot0, ntok, width):
            """dma_gather of ntok tokens in <=MAXTOK pieces (slot-aligned)."""
            done = 0
            while done < ntok:
                take = min(MAXTOK, ntok - done)
                s0 = slot0 + done // 128
                s1 = s0 + (take + 127) // 128
                nc.gpsimd.dma_gather(
                    zg[:, s0:s1, :], src_ap,
                    ipt[:, col0 + done // 16:col0 + (done + take) // 16],
                    take, take, width)
                done += take

        # ---------------- edge phase ----------------
        def edge_phase(layer, src_s, width, er_src, post, mid_cb=None):
            gat = layer < 2
            rw = 264 if gat else width
            with tc.tile_pool(name=f"e{layer}", bufs=2) as pool, \
                 tc.tile_pool(name=f"e{layer}q", bufs=2) as poolq, \
                 tc.tile_pool(name=f"e{layer}r", bufs=1, space="PSUM") as ppr, \
                 tc.tile_pool(name=f"e{layer}x", bufs=1, space="PSUM") as ppx, \
                 tc.tile_pool(name=f"e{layer}p", bufs=2, space="PSUM") as pp:
                for gi, (blocks, k0, k1, slotmap) in enumerate(groups):
                    ks = k0 + k1
                    n0, n1 = k0 * 128, k1 * 128
                    ipw = n0 // 16 + n1 // 16
                    ipt = poolq.tile([128, ipw], I16, tag="ipt")
                    nc.sync.dma_start(
                        out=ipt[:], in_=ip_d[:, ip_off[gi]:ip_off[gi] + ipw])
                    dvt = poolq.tile([128, maxslots], BF16, tag="dvt")
                    nc.sync.dma_start(
                        out=dvt[:, 0:ks],
                        in_=dv_d[:, dv_off[gi]:dv_off[gi] + ks])
                    zg = pool.tile([128, maxslots, width], BF16, tag="zg")
                    if k0:
                        gather_split(zg, src_s[0][:, :], ipt, 0, 0, n0, width)
                    if k1:
                        gather_split(zg, src_s[1][:, :], ipt, n0 // 16, k0,
                                     n1, width)
                    se = pool.tile([128, maxslots, 128], BF16, tag="se")
                    nc.vector.tensor_tensor(
                        out=se[:, 0:ks, :],
                        in0=dvt[:, 0:ks].unsqueeze(2).broadcast_to(
                            [128, ks, 128]),
                        in1=iota[:].unsqueeze(1).broadcast_to([128, ks, 128]),
                        op=ALU.is_equal)
                    if gat:
                        # er[dst] per token: one-hot SE_T x er_blk on PE
                        erb = poolq.tile([128, GB, 4], BF16, tag="erb")
                        nblk_ = len(blocks)
                        nc.sync.dma_start(
                            out=erb[:, 0:nblk_, :],
                            in_=er_src[blocks[0] * 128:
                                       (blocks[0] + nblk_) * 128, :]
                            .rearrange("(t p) f -> p t f", p=128))
                        seT = pool.tile([128, maxslots, 128], BF16, tag="seT")
                        erp = ppr.tile([128, 512], FP32, tag="erp")
                        for slot, (bi, sl) in enumerate(slotmap):
                            ptT = pp.tile([128, 128], BF16, tag="tp1")
                            nc.tensor.transpose(out=ptT[:],
                                                in_=se[:, slot, :],
                                                identity=ident[:])
                            nc.scalar.copy(out=seT[:, slot, :], in_=ptT[:])
                            nc.tensor.matmul(
                                erp[:, 4 * slot:4 * slot + 4],
                                lhsT=seT[:, slot, :], rhs=erb[:, bi, :],
                                start=(slot == 0), stop=(slot == ks - 1))
                        lg = pool.tile([128, maxslots, H], FP32, tag="lg")
                        nc.vector.tensor_tensor(
                            out=lg[:, 0:ks, :], in0=zg[:, 0:ks, 256:260],
                            in1=erp[:, 0:4 * ks].rearrange(
                                "p (s f) -> p s f", f=4),
                            op=ALU.add)
                        lr = pool.tile([128, maxslots, H], FP32, tag="lr")
                        nc.vector.scalar_tensor_tensor(
                            out=lr[:, 0:ks, :], in0=lg[:, 0:ks, :],
                            scalar=0.2, in1=lg[:, 0:ks, :],
                            op0=ALU.mult, op1=ALU.max)
                        wt = pool.tile([128, maxslots, H], BF16, tag="wt")
                        nc.scalar.activation(out=wt[:, 0:ks, :],
                                             in_=lr[:, 0:ks, :], func=AF.Exp)
                        nc.vector.tensor_tensor(
                            out=zg[:, 0:ks, 0:256].rearrange(
                                "p s (h d) -> p s h d", h=H),
                            in0=zg[:, 0:ks, 0:256].rearrange(
                                "p s (h d) -> p s h d", h=H),
                            in1=wt[:, 0:ks, :].unsqueeze(3).broadcast_to(
                                [128, ks, H, D]),
                            op=ALU.mult)
                        nc.vector.tensor_copy(out=zg[:, 0:ks, 256:260],
                                              in_=wt[:, 0:ks, :])
                    nblk = len(blocks)
                    pbs = [ppr.tile([128, 512], FP32, tag=f"rst{bi}",
                                    name=f"rst{bi}")
                           for bi in range(nblk)]
                    first = [True] * nblk
                    last_slot = {}
                    for slot, (bi, sl) in enumerate(slotmap):
                        last_slot[bi] = slot
                    for slot, (bi, sl) in enumerate(slotmap):
                        nc.tensor.matmul(
                            pbs[bi][:, 0:rw],
                            lhsT=se[:, slot, :], rhs=zg[:, slot, 0:rw],
                            start=first[bi], stop=(slot == last_slot[bi]))
                        first[bi] = False
                    for bi, b in enumerate(blocks):
                        post(b, pbs[bi], pool, pp, ppx)
                    if mid_cb is not None and gi in mid_cb:
                        mid_cb[gi]()

        # ---------------- posts ----------------
        def gat_post(layer):
            bias = b0 if layer == 0 else b1

            def post(b, pb, pool, pp, ppx):
                dmax = pool.tile([128, H], FP32, tag="dmax")
                nc.vector.tensor_scalar_max(dmax[:], pb[:, 256:260], 1e-9)
                rec = pool.tile([128, H], FP32, tag="rec")
                nc.vector.reciprocal(rec[:], dmax[:])
                hb = pool.tile([128, HID], FP32, tag="hb")
                nc.vector.tensor_tensor(
                    out=hb[:].rearrange("p (h d) -> p h d", h=H),
                    in0=pb[:, 0:256].rearrange("p (h d) -> p h d", h=H),
                    in1=rec[:].unsqueeze(2).broadcast_to([128, H, D]),
                    op=ALU.mult)
                hb2 = pool.tile([128, HID], FP32, tag="hb2")
                nc.vector.tensor_tensor(out=hb2[:], in0=hb[:], in1=bias[:],
                                        op=ALU.add)
                hf = pool.tile([128, HID], BF16, tag="hf")
                nc.scalar.activation(out=hf[:], in_=hb2[:], func=AF.Relu)
                if layer == 0:
                    # fused GAT1 node: zel1 = h1 @ w1x
                    hT = pool.tile([128, 2, 128], BF16, tag="hT")
                    for k2 in range(2):
                        pt = pp.tile([128, 128], BF16, tag="tp1")
                        nc.tensor.transpose(
                            out=pt[:], in_=hf[:, k2 * 128:(k2 + 1) * 128],
                            identity=ident[:])
                        nc.any.tensor_copy(out=hT[:, k2, :], in_=pt[:])
                    zp = ppx.tile([128, 512], FP32, tag="z1p")
                    for k2 in range(2):
                        nc.tensor.matmul(zp[:, 0:264], lhsT=hT[:, k2, :],
                                         rhs=w1x[:, k2, :],
                                         start=(k2 == 0), stop=(k2 == 1))
                    z1f = pool.tile([128, 264], BF16, tag="z1f")
                    nc.any.tensor_copy(out=z1f[:], in_=zp[:, 0:264])
                    nc.sync.dma_start(out=loc_rows(zel1_loc, b, 264),
                                      in_=z1f[:])
                    nc.sync.dma_start(
                        out=er_cmp[1][b * 128:(b + 1) * 128, :].rearrange(
                            "(t p) f -> p t f", p=128)[:, 0, :],
                        in_=z1f[:, 260:264])
                else:
                    nc.sync.dma_start(out=loc_rows(hcat_loc, b, 256),
                                      in_=hf[:])
            return post

        def gin_post(layer, stats_pb):
            gidx = layer - 2
            w1 = g0w1 if gidx == 0 else g1w1
            epsv = eps0 if gidx == 0 else eps1
            hc_src = hcat_loc if gidx == 0 else h3_loc
            w_in = 384 if gidx == 0 else 256
            kc = w_in // 128

            def post(b, pb, pool, pp, ppx):
                hcin = pool.tile([128, w_in], BF16, tag="hcin")
                nc.sync.dma_start(out=hcin[:], in_=loc_rows(hc_src, b, w_in))
                xc = pool.tile([128, w_in], BF16, tag="xc")
                nc.vector.scalar_tensor_tensor(
                    out=xc[:], in0=hcin[:], scalar=epsv[:],
                    in1=pb[:, 0:w_in], op0=ALU.mult, op1=ALU.add)
                if b == NB - 1:
                    # zero pad nodes 12500..12543 (partitions 84..127)
                    nc.vector.tensor_tensor(
                        out=xc[:], in0=xc[:],
                        in1=padmask[:].broadcast_to([128, w_in]),
                        op=ALU.mult)
                xT = pool.tile([128, 3, 128], BF16, tag="xT")
                for k2 in range(kc):
                    pt = pp.tile([128, 128], BF16, tag="tp2")
                    nc.tensor.transpose(
                        out=pt[:], in_=xc[:, k2 * 128:(k2 + 1) * 128],
                        identity=ident[:])
                    nc.any.tensor_copy(out=xT[:, k2, :], in_=pt[:])
                xp = ppx.tile([128, 512], FP32, tag="x1p")
                for k2 in range(kc):
                    nc.tensor.matmul(xp[:, 0:HID], lhsT=xT[:, k2, :],
                                     rhs=w1[:, k2, :],
                                     start=(k2 == 0), stop=(k2 == kc - 1))
                x1f = pool.tile([128, HID], BF16, tag="x1f")
                nc.any.tensor_copy(out=x1f[:], in_=xp[:, 0:HID])
                nc.vector.tensor_copy(out=x1_sb[:, b, :], in_=x1f[:])
                sq = pool.tile([128, HID], BF16, tag="sq")
                nc.scalar.activation(out=sq[:], in_=xp[:, 0:HID],
                                     func=AF.Square)
                for col, (srct, chk) in enumerate(
                        ((x1f, 0), (x1f, 1), (sq, 0), (sq, 1))):
                    nc.tensor.matmul(
                        stats_pb[:, col:col + 1],
                        lhsT=srct[:, chk * 128:(chk + 1) * 128], rhs=onesc[:],
                        start=(b == 0 and col == 0),
                        stop=(b == NB - 1 and col == 3))
            return post

        def gin_finish(layer):
            gidx = layer - 2
            w2 = g0w2 if gidx == 0 else g1w2
            pre = "g0" if gidx == 0 else "g1"
            out_f32 = gidx == 1
            with tc.tile_pool(name=f"f{layer}", bufs=3) as pool, \
                 tc.tile_pool(name=f"f{layer}p", bufs=2, space="PSUM") as pp:
                art = pool.tile([128, 4], FP32, tag="art")
                nc.sync.dma_start(out=art[:], in_=arb_out[gidx][:, :])
                mu = pool.tile([128, 2], FP32, tag="mu")
                nc.vector.tensor_scalar_mul(mu[:], art[:, 0:2], 1.0 / N)
                msq = pool.tile([128, 2], FP32, tag="msq")
                nc.vector.tensor_scalar_mul(msq[:], art[:, 2:4], 1.0 / N)
                mu2 = pool.tile([128, 2], FP32, tag="mu2")
                nc.vector.tensor_mul(mu2[:], mu[:], mu[:])
                var = pool.tile([128, 2], FP32, tag="var")
                nc.vector.tensor_sub(var[:], msq[:], mu2[:])
                vare = pool.tile([128, 2], FP32, tag="vare")
                nc.vector.tensor_scalar_add(vare[:], var[:], BN_EPS)
                sd = pool.tile([128, 2], FP32, tag="sd")
                nc.scalar.activation(out=sd[:], in_=vare[:], func=AF.Sqrt)
                rsd = pool.tile([128, 2], FP32, tag="rsd")
                nc.vector.reciprocal(rsd[:], sd[:])
                scl4 = pool.tile([128, 4], FP32, tag="scl4")
                nc.vector.tensor_mul(scl4[:, 0:2], rsd[:],
                                     vec[pre + "g1"][:, :, 0])
                mus = pool.tile([128, 2], FP32, tag="mus")
                nc.vector.tensor_mul(mus[:], mu[:], scl4[:, 0:2])
                nc.vector.tensor_sub(scl4[:, 2:4], vec[pre + "be1"][:, :, 0],
                                     mus[:])
                # broadcast feature-major [128, 4] -> token-major [128, 256]
                ptT = pp.tile([4, 128], FP32, tag="sclTp")
                nc.tensor.transpose(out=ptT[:], in_=scl4[:], identity=identf[:])
                scr = pool.tile([4, 128], FP32, tag="scr")
                nc.any.tensor_copy(out=scr[:], in_=ptT[:])
                # roundtrip rows through DRAM to land each at partition 0
                nc.sync.dma_start(out=scl_dram[gidx][:, :], in_=scr[:])
                sclT = pool.tile([128, HID], FP32, tag="ssclT")
                shfT = pool.tile([128, HID], FP32, tag="sshfT")
                for row, dstt in ((0, sclT), (1, sclT), (2, shfT), (3, shfT)):
                    chk = row % 2
                    srow = pool.tile([1, 128], FP32, tag=f"srow{row}",
                                     name=f"srow{row}")
                    nc.sync.dma_start(out=srow[:],
                                      in_=scl_dram[gidx][row:row + 1, :])
                    bp = pp.tile([128, 128], FP32, tag="bp")
                    nc.tensor.matmul(bp[:], lhsT=onesr[:, :], rhs=srow[:],
                                     start=True, stop=True)
                    nc.any.tensor_copy(out=dstt[:, chk * 128:(chk + 1) * 128],
                                       in_=bp[:])
                # pass B over x1_sb; 7-block tiles stay within row halves
                passb_tiles = list(range(0, NB, 7))
                for t0 in passb_tiles:
                    x1n = pool.tile([128, 7, HID], BF16, tag="x1n")
                    nc.vector.tensor_tensor(
                        out=x1n[:], in0=x1_sb[:, t0:t0 + 7, :],
                        in1=sclT[:].unsqueeze(1).broadcast_to([128, 7, HID]),
                        op=ALU.mult)
                    nc.vector.tensor_tensor(
                        out=x1n[:], in0=x1n[:],
                        in1=shfT[:].unsqueeze(1).broadcast_to([128, 7, HID]),
                        op=ALU.add)
                    nc.scalar.activation(out=x1n[:], in_=x1n[:], func=AF.Relu)
                    ho = pool.tile([128, 7, HID], FP32 if out_f32 else BF16,
                                   tag="ho")
                    for t in range(7):
                        xT = pool.tile([128, 2, 128], BF16, tag="xT2")
                        for k2 in range(2):
                            pt2 = pp.tile([128, 128], BF16, tag="tp3")
                            nc.tensor.transpose(
                                out=pt2[:],
                                in_=x1n[:, t, k2 * 128:(k2 + 1) * 128],
                                identity=ident[:])
                            nc.any.tensor_copy(out=xT[:, k2, :], in_=pt2[:])
                        x2p = pp.tile([128, 512], FP32, tag="x2p")
                        for k2 in range(2):
                            nc.tensor.matmul(x2p[:, 0:HID], lhsT=xT[:, k2, :],
                                             rhs=w2[:, k2, :],
                                             start=(k2 == 0), stop=(k2 == 1))
                        hb3 = pool.tile([128, HID], FP32, tag="hb3")
                        nc.vector.tensor_tensor(out=hb3[:], in0=x2p[:, 0:HID],
                                                in1=b2t[pre + "b2t"][:],
                                                op=ALU.add)
                        nc.scalar.activation(out=ho[:, t, :], in_=hb3[:],
                                             func=AF.Relu)
                    if out_f32:
                        nc.sync.dma_start(out=_rows(out_d, t0, 7, HID),
                                          in_=ho[:])
                    else:
                        half, tt = (0, t0) if t0 < HB else (1, t0 - HB)
                        nc.sync.dma_start(
                            out=_rows(h3_loc[half], tt, 7, HID), in_=ho[:])
                        if t0 + 7 == HB:
                            allgather_half(h3_loc, h3_s, 0)
                        elif t0 + 7 == NB:
                            allgather_half(h3_loc, h3_s, 1)

        def allgather_half(src_halves, dsts, half):
            nc.gpsimd.collective_compute(
                "AllGather", ALU.bypass, replica_groups=RGROUPS,
                ins=[src_halves[half][:, :].opt()],
                outs=[dsts[half][:, :].opt()])

        def allreduce_stats(gidx, stats_pb):
            with tc.tile_pool(name=f"ar{gidx}", bufs=1) as pool:
                arp = pool.tile([128, 4], FP32, tag="arp")
                nc.vector.tensor_copy(out=arp[:], in_=stats_pb[:, 0:4])
                nc.sync.dma_start(out=arb_in[gidx][:, :], in_=arp[:])
            nc.gpsimd.collective_compute(
                "AllReduce", ALU.add, replica_groups=RGROUPS,
                ins=[arb_in[gidx][:, :].opt()],
                outs=[arb_out[gidx][:, :].opt()])

        # ---------------- schedule ----------------
        # half 0 = dst blocks 0..48; group 12 = blocks 48..51 completes it
        gat0_node()
        hcat_prefill()
        er_fill()
        if STAGES >= 2:
            edge_phase(0, zel0_s, ZW, er_cmp[0], gat_post(0),
                       mid_cb={12: lambda: allgather_half(zel1_loc, zel1_s, 0),
                               24: lambda: allgather_half(zel1_loc, zel1_s, 1)})
        if STAGES >= 3:
            edge_phase(1, zel1_s, ZW, er_cmp[1], gat_post(1),
                       mid_cb={12: lambda: allgather_half(hcat_loc, hcat_s, 0),
                               24: lambda: allgather_half(hcat_loc, hcat_s, 1)})
        if STAGES >= 4:
            with tc.tile_pool(name="sp0", bufs=1, space="PSUM") as sp:
                stats0 = sp.tile([128, 512], FP32, tag="stats0")
                edge_phase(2, hcat_s, ZW, None, gin_post(2, stats0))
                allreduce_stats(0, stats0)
            gin_finish(2)
        if STAGES >= 5:
            with tc.tile_pool(name="sp1", bufs=1, space="PSUM") as sp:
                stats1 = sp.tile([128, 512], FP32, tag="stats1")
                edge_phase(3, h3_s, HID, None, gin_post(3, stats1))
                allreduce_stats(1, stats1)
            gin_finish(3)

        big.release()
        cst.release()

    nc.compile()
    return nc


_CACHE = {}


def kernel(**inputs):
    in_maps, plan = _preprocess(inputs)
    nc = _CACHE.get(plan[0])
    if nc is None:
        nc = build_program(plan)
        _CACHE[plan[0]] = nc
    res = run_bass_kernel_spmd(nc, in_maps, core_ids=list(range(8)))
    out = np.zeros((N, T * HID), np.float32)
    for c in range(8):
        q, r = c // P, c % P
        out[r * NQ:(r + 1) * NQ, q * HID:(q + 1) * HID] = \
            np.asarray(res.results[c]["out"], np.float32)[:NQ]
    return out
